# revision 9
# baseline (speedup 1.0000x reference)
"""Trainium2 Bass kernel for a dense pre-norm transformer block.

Problem: x[8, 1024, 768]; per-batch-element transformer block
  (LN1 -> qkv -> 12-head attention -> proj residual -> LN2 -> MLP(gelu) residual).

Strategy (v2, fp8):
  - Pure data-parallel: 8 NeuronCores, one batch element each. No collectives.
  - Activations channel-major ("T layout", [C, tokens]); host transposes.
  - fp8e4(+DoubleRow, 2 k-tiles per matmul) for qkv/V/AV/proj/fc2 GEMMs;
    fc1 stays bf16 (dominant error site); scores bf16 with PE row-group
    concurrency; residual stream + LN stats fp32/f32r.
  - All weights SBUF-resident (~72KB/partition), loaded once, zero
    steady-state weight DMA.
  - LN gains folded into the following weights, LN biases folded into the
    following biases (host-side). Normalize = sub+mul only. rstd computed as
    exp(-0.5*ln(var+eps)) so ACT stays on the ln/exp table set through
    LN+attention; token-half pipelining hides the stat chain.
  - Softmax denominators ride a ones-column in V (also fp8/DoubleRow);
    per-head-pair normalization broadcast via small DRAM round-trip.
"""

import ml_dtypes
import numpy as np

import concourse.bacc as bacc
import concourse.bass as bass
import concourse.mybir as mybir
from concourse import tile
from concourse.bass_utils import run_bass_kernel_spmd

AF = mybir.ActivationFunctionType
ALU = mybir.AluOpType
PM = mybir.MatmulPerfMode
f32 = mybir.dt.float32
f32r = mybir.dt.float32r
bf16 = mybir.dt.bfloat16
f8 = mybir.dt.float8e4

P = 128
DIM = 768
CT = DIM // P            # 6 channel tiles
KP = CT // 2             # 3 channel-tile pairs (DoubleRow)
N = 1024                 # tokens
NT = N // P              # 8 token tiles
JP = NT // 2             # 4 key-tile pairs
NH = 12                  # heads
DH = 64                  # head dim
VW = 80                  # padded V row width (DH + ones col, 16B aligned)
HID = 3072
HT = HID // P            # 24 hidden tiles
HP = HT // 2             # 12 hidden-tile pairs
B = 8
EPS = 1e-5
SCALE = DH ** -0.5
SW = 16.0                # fp8 weight scale (qkv/v/fc2)
ISW = 1.0 / SW


def _t6(dram_2d):
    return dram_2d.rearrange("(a p) m -> p a m", p=P)


def build_nc(reps=1):
    nc = bacc.Bacc("TRN2", target_bir_lowering=False, debug=False)

    # ---- I/O ----
    xT = nc.dram_tensor("xT", [DIM, N], f32r, kind="ExternalInput")
    wqk = nc.dram_tensor("wqk", [P, 12 * KP * 2 * P], f8, kind="ExternalInput")
    wv = nc.dram_tensor("wv", [P, KP * 2 * DIM], f8, kind="ExternalInput")
    wproj = nc.dram_tensor("wproj", [P, KP * 2 * DIM], f8, kind="ExternalInput")
    wfc1 = nc.dram_tensor("wfc1", [P, HT * CT * P], bf16, kind="ExternalInput")
    wfc2 = nc.dram_tensor("wfc2", [P, HP * 2 * DIM], f8, kind="ExternalInput")
    bqk = nc.dram_tensor("bqk", [P, 12], f32, kind="ExternalInput")
    bv = nc.dram_tensor("bv", [DIM], f32, kind="ExternalInput")
    bproj = nc.dram_tensor("bproj", [P, CT], f32, kind="ExternalInput")
    bfc1 = nc.dram_tensor("bfc1", [P, HT], f32, kind="ExternalInput")
    bfc2 = nc.dram_tensor("bfc2", [P, CT], f32, kind="ExternalInput")
    outT = nc.dram_tensor("outT", [DIM, N], f32r, kind="ExternalOutput")

    args = locals()
    with tile.TileContext(nc) as tc:
        _body(nc, tc, args, reps)
    nc.compile()
    return nc


def _body(nc, tc, t, reps=1):
    xT, outT = t["xT"], t["outT"]

    with (
        tc.tile_pool(name="const", bufs=1) as const,
        tc.tile_pool(name="work", bufs=1) as work,
        tc.tile_pool(name="dram", bufs=1, space="DRAM") as dram,
    ):
        # ---- SBUF-resident weights (loaded once) ----
        wqk_sb = const.tile([P, 12, KP, 2, P], f8)
        nc.sync.dma_start(wqk_sb[:].rearrange("p a b c d -> p (a b c d)"),
                          t["wqk"][:])
        wv_sb = const.tile([P, KP, 2, DIM], f8)
        nc.sync.dma_start(wv_sb[:].rearrange("p a b c -> p (a b c)"),
                          t["wv"][:])
        wp_sb = const.tile([P, KP, 2, DIM], f8)
        nc.sync.dma_start(wp_sb[:].rearrange("p a b c -> p (a b c)"),
                          t["wproj"][:])
        w1_sb = const.tile([P, HT, CT, P], bf16)
        nc.scalar.dma_start(w1_sb[:].rearrange("p a b c -> p (a b c)"),
                            t["wfc1"][:])
        w2_sb = const.tile([P, HP, 2, DIM], f8)
        nc.sync.dma_start(w2_sb[:].rearrange("p a b c -> p (a b c)"),
                          t["wfc2"][:])

        # ---- constants ----
        ones_ln = const.tile([P, P], f32)
        nc.vector.memset(ones_ln[:], 1.0 / DIM)
        ones_r = const.tile([P, P], f32r)
        nc.scalar.copy(ones_r[:], ones_ln[:])
        eps_t = const.tile([P, 1], f32)
        nc.vector.memset(eps_t[:], EPS)
        bqk_sb = const.tile([P, 12], f32)
        nc.sync.dma_start(bqk_sb[:], t["bqk"][:])
        bproj_sb = const.tile([P, CT], f32)
        nc.sync.dma_start(bproj_sb[:], t["bproj"][:])
        bfc1_sb = const.tile([P, HT], f32)
        nc.sync.dma_start(bfc1_sb[:], t["bfc1"][:])
        bfc2_sb = const.tile([P, CT], f32)
        nc.sync.dma_start(bfc2_sb[:], t["bfc2"][:])
        vb_sb = const.tile([P, DIM], f32)
        bv_ap = t["bv"][:]
        nc.gpsimd.dma_start(
            vb_sb[:],
            bass.AP(tensor=bv_ap.tensor, offset=bv_ap.offset,
                    ap=[[0, P], [1, DIM]]))

        # ---- persistent activations ----
        xsb = const.tile([P, CT, N], f32r)        # residual stream
        for ct in range(CT):
            nc.sync.dma_start(xsb[:, ct, :], xT[ct * P:(ct + 1) * P, :])
        h1 = const.tile([P, CT, N], f8)           # LN1 out
        h2 = const.tile([P, CT, N], bf16)         # LN2 out
        h3 = const.tile([P, HT, N], f8)           # gelu(fc1) out
        ob = const.tile([P, CT, N], bf16)         # unnormalized attn out
        o8 = const.tile([P, CT, N], f8)           # normalized attn out
        vsb = const.tile([P, NH, JP, 2, VW], f8)  # V + ones col, padded
        with nc.allow_low_precision(reason="ones column exact in fp8"):
            nc.vector.memset(vsb[:, :, :, :, DH:DH + 1], 1.0)
        dscr = dram.tile([NH, N], bf16)

        def layer_norm_T(src, dst):
            """dst = (src - mu) * rstd, per token-half; dst is fp8/bf16."""
            with (
                tc.tile_pool(name="ln_tmp", bufs=1) as tmp,
                tc.tile_pool(name="ln_ps", bufs=1, space="PSUM") as lps,
            ):
                for hh in range(2):
                    sl = bass.ts(hh, 512)
                    mu_ps = lps.tile([P, 512], f32, tag="mups", bufs=2,
                                     name="mu_ps")
                    e2_ps = lps.tile([P, 512], f32, tag="e2ps", bufs=2,
                                     name="e2_ps")
                    for ct in range(CT):
                        sq = tmp.tile([P, 512], f32r, tag="sq", bufs=3,
                                      name="sq")
                        eng = nc.gpsimd if ct % 2 else nc.vector
                        eng.tensor_mul(sq[:], src[:, ct, sl], src[:, ct, sl])
                        nc.tensor.matmul(
                            mu_ps[:], ones_r[:], src[:, ct, sl],
                            start=(ct == 0), stop=(ct == CT - 1))
                        nc.tensor.matmul(
                            e2_ps[:], ones_r[:], sq[:],
                            start=(ct == 0), stop=(ct == CT - 1))
                    mu_sb = tmp.tile([P, 512], f32, tag="musb", bufs=2,
                                     name="mu_sb")
                    nc.vector.tensor_copy(mu_sb[:], mu_ps[:])
                    mu2 = tmp.tile([P, 512], f32, tag="mu2", bufs=2,
                                   name="mu2")
                    nc.vector.tensor_mul(mu2[:], mu_sb[:], mu_sb[:])
                    var = tmp.tile([P, 512], f32, tag="var", bufs=2,
                                   name="var")
                    nc.vector.tensor_sub(var[:], e2_ps[:], mu2[:])
                    # rstd = exp(-0.5 * ln(var + eps)): stays on act set 6
                    lnv = tmp.tile([P, 512], f32, tag="lnv", bufs=2,
                                   name="lnv")
                    nc.scalar.activation(lnv[:], var[:], AF.Ln,
                                         bias=eps_t[:], scale=1.0)
                    rstd = tmp.tile([P, 512], f32, tag="rstd", bufs=2,
                                    name="rstd")
                    nc.scalar.activation(rstd[:], lnv[:], AF.Exp, scale=-0.5)
                    for ct in range(CT):
                        e_sub = nc.gpsimd if ct % 2 == 0 else nc.vector
                        e_mul = nc.gpsimd if ct in (1, 3) else nc.vector
                        t1 = tmp.tile([P, 512], f32, tag="t1", bufs=4,
                                      name="t1")
                        e_sub.tensor_sub(t1[:], src[:, ct, sl], mu_sb[:])
                        with nc.allow_low_precision(reason="ln out fp8"):
                            e_mul.tensor_mul(dst[:, ct, sl], t1[:], rstd[:])

        for _rep in range(reps):
            # ======== LN1 ========
            layer_norm_T(xsb, h1)

            # ==== QKV production (upfront), then exp-saturated attention ====
            with tc.tile_pool(name="att_sb", bufs=1) as asb:
              with tc.tile_pool(name="qkv_ps", bufs=1, space="PSUM") as qps:
                qk_all = asb.tile([P, 12, N], bf16, name="qk_all")
                for tp in range(CT):
                    for mt in (tp, CT + tp):
                        qkps = qps.tile([P, N], f32, tag="qkps", bufs=2,
                                        name="qkps")
                        for hh in range(2):
                            sl = bass.ts(hh, 512)
                            for kp in range(KP):
                                nc.tensor.matmul(
                                    qkps[:, sl],
                                    wqk_sb[:, mt, kp, :, :],
                                    h1[:, 2 * kp:2 * kp + 2, sl],
                                    start=(kp == 0), stop=(kp == KP - 1),
                                    perf_mode=PM.DoubleRow)
                        with nc.allow_low_precision(reason="qk bf16"):
                            nc.scalar.activation(
                                qk_all[:, mt, :], qkps[:], AF.Identity,
                                bias=bqk_sb[:, mt:mt + 1], scale=ISW)
                for it in range(NT):
                    vps = qps.tile([P, DIM], f32, tag="vps", bufs=2,
                                   name="vps")
                    for c0, cn in ((0, 512), (512, 256)):
                        for kp in range(KP):
                            nc.tensor.matmul(
                                vps[:, c0:c0 + cn],
                                h1[:, 2 * kp:2 * kp + 2,
                                   it * P:(it + 1) * P],
                                wv_sb[:, kp, :, c0:c0 + cn],
                                start=(kp == 0), stop=(kp == KP - 1),
                                perf_mode=PM.DoubleRow)
                    with nc.allow_low_precision(reason="v fp8"):
                        nc.vector.scalar_tensor_tensor(
                            out=vsb[:, :, it // 2, it % 2, 0:DH],
                            in0=vps[:].rearrange("p (h d) -> p h d", d=DH),
                            scalar=ISW, op0=ALU.mult,
                            in1=vb_sb[:].rearrange("p (h d) -> p h d", d=DH),
                            op1=ALU.add)

              with tc.tile_pool(name="att_ps", bufs=1,
                                space="PSUM") as aps:
                    def attn_jp(tp, jp, av0, av1):
                        """Scores+exp per head for key tiles 2jp,2jp+1; AV
                        via DoubleRow over the key-tile pair."""
                        qt = qk_all[:, tp, :]
                        kt2 = qk_all[:, CT + tp, :]
                        e0 = asb.tile([P, 2, N], f8, tag="e0", bufs=2,
                                      name="e0")
                        e1 = asb.tile([P, 2, N], f8, tag="e1", bufs=2,
                                      name="e1")
                        for j2 in range(2):
                            jt = 2 * jp + j2
                            js = slice(jt * P, (jt + 1) * P)
                            sc0 = aps.tile([P, N], f32, tag="sc", bufs=2,
                                           name="sc0")
                            for hh in range(2):
                                sl = bass.ts(hh, 512)
                                nc.tensor.matmul(
                                    sc0[:, sl], kt2[0:DH, js],
                                    qt[0:DH, sl], tile_position=(0, 0))
                            with nc.allow_low_precision(reason="exp fp8"):
                                nc.scalar.activation(e0[:, j2, :], sc0[:],
                                                     AF.Exp, scale=SCALE)
                            sc1 = aps.tile([P, N], f32, tag="sc", bufs=2,
                                           name="sc1")
                            for hh in range(2):
                                sl = bass.ts(hh, 512)
                                nc.tensor.matmul(
                                    sc1[:, sl], kt2[DH:P, js],
                                    qt[DH:P, sl], tile_position=(DH, 0))
                            with nc.allow_low_precision(reason="exp fp8"):
                                nc.scalar.activation(e1[:, j2, :], sc1[:],
                                                     AF.Exp, scale=SCALE)
                        for hh in range(2):
                            sl = bass.ts(hh, 512)
                            nc.tensor.matmul(
                                av0[:, sl], vsb[:, 2 * tp, jp, :, 0:DH + 1],
                                e0[:, :, sl],
                                start=(jp == 0), stop=(jp == JP - 1),
                                perf_mode=PM.DoubleRow)
                            nc.tensor.matmul(
                                av1[:, sl],
                                vsb[:, 2 * tp + 1, jp, :, 0:DH + 1],
                                e1[:, :, sl],
                                start=(jp == 0), stop=(jp == JP - 1),
                                perf_mode=PM.DoubleRow)

                    def finish_pair(tp, av0, av1):
                        # evict unnormalized o^T + denominators -> DRAM
                        nc.vector.tensor_copy(ob[0:DH, tp, :], av0[0:DH, :])
                        te = asb.tile([DH + 1, N], bf16, tag="tmpo", bufs=2,
                                      name="te")
                        nc.vector.tensor_copy(te[DH:DH + 1, :],
                                              av0[DH:DH + 1, :])
                        nc.sync.dma_start(dscr[2 * tp, :], te[DH:DH + 1, :])
                        to = asb.tile([DH + 1, N], bf16, tag="tmpo", bufs=2,
                                      name="to")
                        nc.vector.tensor_copy(to[:], av1[:])
                        nc.sync.dma_start(ob[DH:P, tp, :], to[0:DH, :])
                        nc.sync.dma_start(dscr[2 * tp + 1, :],
                                          to[DH:DH + 1, :])
                        # normalize: Rt = 1/denoms broadcast across partitions
                        Rt = asb.tile([P, N], bf16, tag="Rt", bufs=2,
                                      name="Rt")
                        for hh in range(2):
                            srcb = bass.AP(
                                tensor=dscr.tensor,
                                offset=dscr.offset + (2 * tp + hh) * N,
                                ap=[[0, DH], [1, N]])
                            nc.gpsimd.dma_start(Rt[hh * DH:(hh + 1) * DH, :],
                                                srcb)
                        with nc.allow_low_precision(reason="softmax denom"):
                            nc.vector.reciprocal(Rt[:], Rt[:])
                            nc.vector.tensor_mul(o8[:, tp, :], ob[:, tp, :],
                                                 Rt[:])

                    for tp in range(CT):
                        av0 = aps.tile([DH + 1, N], f32, tag="av", bufs=2,
                                       name="av0")
                        av1 = aps.tile([DH + 1, N], f32, tag="av", bufs=2,
                                       name="av1")
                        for jp in range(JP):
                            attn_jp(tp, jp, av0, av1)
                        finish_pair(tp, av0, av1)

            # ======== proj + residual ========
            with tc.tile_pool(name="pj_ps", bufs=1, space="PSUM") as pps:
                for mt in range(CT):
                    for hh in range(2):
                        sl = bass.ts(hh, 512)
                        ps = pps.tile([P, 512], f32, tag="ps", bufs=6,
                                      name="ps")
                        for kp in range(KP):
                            nc.tensor.matmul(
                                ps[:],
                                wp_sb[:, kp, :, mt * P:(mt + 1) * P],
                                o8[:, 2 * kp:2 * kp + 2, sl],
                                start=(kp == 0), stop=(kp == KP - 1),
                                perf_mode=PM.DoubleRow)
                        nc.vector.scalar_tensor_tensor(
                            out=xsb[:, mt, sl], in0=ps[:],
                            scalar=bproj_sb[:, mt:mt + 1], op0=ALU.add,
                            in1=xsb[:, mt, sl], op1=ALU.add)

            # ======== LN2 + MLP ========
            layer_norm_T(xsb, h2)
            with tc.tile_pool(name="f1_ps", bufs=1, space="PSUM") as f1p:
                for hh in range(2):
                    sl = bass.ts(hh, 512)
                    for ct in range(HT):
                        h3ps = f1p.tile([P, 512], f32, tag="h3ps", bufs=6,
                                        name="h3ps")
                        for kt in range(CT):
                            nc.tensor.matmul(
                                h3ps[:], w1_sb[:, ct, kt, :],
                                h2[:, kt, sl],
                                start=(kt == 0), stop=(kt == CT - 1))
                        with nc.allow_low_precision(reason="h3 fp8"):
                            nc.scalar.activation(
                                h3[:, ct, sl], h3ps[:], AF.Gelu,
                                bias=bfc1_sb[:, ct:ct + 1], scale=1.0)
            with (
                tc.tile_pool(name="f2_ps", bufs=1, space="PSUM") as f2p,
                tc.tile_pool(name="f2_sb", bufs=1) as f2s,
            ):
                for g in range(2):  # output groups: mt 0-2, 3-5
                    f2ps = [f2p.tile([P, N], f32, tag=f"f2_{i}", bufs=1,
                                     name=f"f2ps{i}") for i in range(3)]
                    for hp in range(HP):
                        for i in range(3):
                            mc = g * 384 + i * P
                            for hh in range(2):
                                sl = bass.ts(hh, 512)
                                nc.tensor.matmul(
                                    f2ps[i][:, sl],
                                    w2_sb[:, hp, :, mc:mc + P],
                                    h3[:, 2 * hp:2 * hp + 2, sl],
                                    start=(hp == 0), stop=(hp == HP - 1),
                                    perf_mode=PM.DoubleRow)
                    for i in range(3):
                        mt = g * 3 + i
                        # t = psum/SW + bfc2 (ACT), then xsb += t (DVE)
                        tt = f2s.tile([P, N], f32, tag="tt", bufs=3,
                                      name="tt")
                        nc.scalar.activation(
                            tt[:], f2ps[i][:], AF.Identity,
                            bias=bfc2_sb[:, mt:mt + 1], scale=ISW)
                        nc.vector.tensor_add(xsb[:, mt, :], xsb[:, mt, :],
                                             tt[:])
                        if _rep == reps - 1:
                            nc.sync.dma_start(
                                _t6(outT)[:, mt, :], xsb[:, mt, :])


_NC_CACHE = None


def _get_nc():
    global _NC_CACHE
    if _NC_CACHE is None:
        _NC_CACHE = build_nc()
    return _NC_CACHE


def _prep_shared(qkv_w, qkv_b, proj_w, proj_b, fc1_w, fc1_b, fc2_w, fc2_b,
                 ln1_g, ln1_b, ln2_g, ln2_b):
    c = lambda a: np.ascontiguousarray(np.asarray(a, dtype=np.float32))
    f8np = ml_dtypes.float8_e4m3
    qkv_w = np.asarray(qkv_w, np.float32)
    fc1_w = np.asarray(fc1_w, np.float32)
    fc2_w = np.asarray(fc2_w, np.float32)
    proj_w = np.asarray(proj_w, np.float32)
    # fold LN gains into weights, LN biases into following biases
    wqkv_g = np.asarray(ln1_g, np.float32)[:, None] * qkv_w
    bqkv_eff = np.asarray(qkv_b, np.float32) + \
        np.asarray(ln1_b, np.float32) @ qkv_w
    wfc1_g = np.asarray(ln2_g, np.float32)[:, None] * fc1_w
    bfc1_eff = np.asarray(fc1_b, np.float32) + \
        np.asarray(ln2_b, np.float32) @ fc1_w

    # wqk: [P, 12, KP, 2, P]; m-tiles 0-5 = q, 6-11 = k; SW-scaled fp8
    wqk_h = (wqkv_g[:, :2 * DIM] * SW).reshape(KP, 2, P, 12, P)
    wqk_h = wqk_h.transpose(2, 3, 0, 1, 4).reshape(P, 12 * KP * 2 * P)
    # wv: [P, KP, 2, DIM]
    wv_h = (wqkv_g[:, 2 * DIM:] * SW).reshape(KP, 2, P, DIM)
    wv_h = wv_h.transpose(2, 0, 1, 3).reshape(P, KP * 2 * DIM)
    # wproj: [P, KP, 2, DIM] (unscaled)
    wp_h = proj_w.reshape(KP, 2, P, DIM).transpose(2, 0, 1, 3).reshape(
        P, KP * 2 * DIM)
    # wfc1: [P, HT, CT, P] bf16 (g2-folded)
    w1_h = wfc1_g.reshape(CT, P, HT, P).transpose(1, 2, 0, 3).reshape(
        P, HT * CT * P)
    # wfc2: [P, HP, 2, DIM] fp8, SW-scaled
    w2_h = (fc2_w * SW).reshape(HP, 2, P, DIM).transpose(2, 0, 1, 3).reshape(
        P, HP * 2 * DIM)
    return {
        "wqk": np.ascontiguousarray(wqk_h).astype(f8np),
        "wv": np.ascontiguousarray(wv_h).astype(f8np),
        "wproj": np.ascontiguousarray(wp_h).astype(f8np),
        "wfc1": np.ascontiguousarray(w1_h).astype(ml_dtypes.bfloat16),
        "wfc2": np.ascontiguousarray(w2_h).astype(f8np),
        "bqk": c(bqkv_eff[:2 * DIM].reshape(12, P).T),
        "bv": c(bqkv_eff[2 * DIM:]),
        "bproj": c(np.asarray(proj_b).reshape(CT, P).T),
        "bfc1": c(bfc1_eff.reshape(HT, P).T),
        "bfc2": c(np.asarray(fc2_b).reshape(CT, P).T),
    }


def run(x, shared, **spmd_kwargs):
    nc = _get_nc()
    x = np.asarray(x, dtype=np.float32)
    in_maps = [
        {**shared, "xT": np.ascontiguousarray(x[b].T)} for b in range(B)
    ]
    res = run_bass_kernel_spmd(nc, in_maps, core_ids=list(range(B)),
                               **spmd_kwargs)
    out = np.stack([res.results[b]["outT"].T for b in range(B)])
    return out.astype(np.float32), res


def kernel(x, ln1_g, ln1_b, qkv_w, qkv_b, proj_w, proj_b,
           ln2_g, ln2_b, fc1_w, fc1_b, fc2_w, fc2_b):
    shared = _prep_shared(qkv_w, qkv_b, proj_w, proj_b, fc1_w, fc1_b,
                          fc2_w, fc2_b, ln1_g, ln1_b, ln2_g, ln2_b)
    out, _ = run(x, shared)
    return out
